# revision 3
# baseline (speedup 1.0000x reference)
"""Chamfer loss Trainium2 kernel.

Per-sample Chamfer loss over (bs=8, n=4096, d=3) point clouds, data-parallel
over the batch axis: one sample per NeuronCore, no cross-core communication.

Math: dist[i,j] = sqrt(eps + relu(||y_i||^2 + ||x_j||^2 - 2 y_i.x_j)).
sqrt(eps + relu(.)) is monotonic, so min-reduce the *squared* matrix and apply
the transform to the reduced 4096-vectors only.

The squared-distance matrix is produced on the TensorEngine as a single K=24
bf16 matmul per tile: y/x are split hi+lo in bf16 (y ~ y0+y1), the squared
norms into three bf16 addends, and all product terms are stacked along the
contraction axis. PSUM accumulates in fp32, giving |sq - exact| ~ 3e-4, i.e.
~1e-5 relative error on the final loss. bf16 streams 1 cycle/row vs fp32's 4.

Per 128-row block (32 of them):
  PE    : 8 matmuls (N=512) into two [128,2048] fp32 PSUM tiles
  ACT   : copies each PSUM tile to an SBUF bf16 strip (frees PSUM, enables
          2x/4x-rate bf16 DVE ops)
  DVE   : running column-min (elementwise bf16 tensor_tensor min into
          colacc[128,4096], 2x mode) and row-min via a bf16 tensor_tensor
          min fold chain 4096->256 (2x mode) + one 1x-rate reduce
          (tensor_scalar's min-accumulate measures 1x on HW, so folds win)
Epilogue: colacc partition-min via PE transpose + DVE min-reduce per 128-col
chunk, then relu/+eps/sqrt on the two [128,32] min matrices, sum-reduce, a
ones-vector matmul for the partition sum, scale by 1/4096.

The input DMA is issued in geometric chunks (first 256 cols, then 256, 512,
1024, 2048) so the first matmul's operands land ~5us sooner than with
quarter-split DMAs; the whole kernel is start-latency + DVE-busy bound.
"""

import os
import sys
import functools

for _p in ("/opt/trn_rl_repo", "/root/.axon_site/_ro/trn_rl_repo"):
    if os.path.isdir(_p) and _p not in sys.path:
        sys.path.insert(0, _p)

import numpy as np
import ml_dtypes

import concourse.bass as bass
import concourse.bacc as bacc
import concourse.mybir as mybir
import concourse.tile as tile
from concourse import bass_utils

BF16 = ml_dtypes.bfloat16
F32 = np.float32

N = 4096          # points per cloud
P = 128           # partitions
NB = N // P       # 32 row blocks
H = 2048          # strip width (half of N), 4 PSUM banks
K = 24            # stacked contraction rows
MM_N = 512        # moving free dim per matmul (TRN2 ISA cap)
EPS = 1e-6
BIG = 1e30

AF = mybir.ActivationFunctionType
ALU = mybir.AluOpType
AX = mybir.AxisListType
DT = mybir.dt



def _emit(nc):
    lhsT_d = nc.dram_tensor("lhst_in", [K, N], DT.bfloat16, kind="ExternalInput")
    rhs_d = nc.dram_tensor("rhs_in", [K, N], DT.bfloat16, kind="ExternalInput")
    ident_d = nc.dram_tensor("ident_in", [P, P], DT.bfloat16, kind="ExternalInput")
    out_d = nc.dram_tensor("loss_out", [1, 1], DT.float32, kind="ExternalOutput")

    with tile.TileContext(nc) as tc:
        with (
            tc.tile_pool(name="const", bufs=1) as cpool,
            tc.tile_pool(name="strip", bufs=2) as spool,
            tc.tile_pool(name="scr", bufs=2) as scrpool,
            tc.tile_pool(name="psum", bufs=2, space="PSUM") as ppool,
        ):
            lhsT = cpool.tile([K, N], DT.bfloat16, tag="lhsT")
            rhs = cpool.tile([K, N], DT.bfloat16, tag="rhs")
            ident = cpool.tile([P, P], DT.bfloat16, tag="ident")
            colacc = cpool.tile([P, N], DT.bfloat16, tag="colacc")
            rowacc = cpool.tile([P, NB], DT.float32, tag="rowacc")
            colminT = cpool.tile([P, NB], DT.float32, tag="colminT")
            ones = cpool.tile([P, 1], DT.float32, tag="ones")
            epsc = cpool.tile([P, 1], DT.float32, tag="epsc")

            # geometric input DMA chunks: the first matmul only needs
            # lhsT[:, 0:128] and rhs[:, 0:512], so land a small chunk first
            bounds = [0, 256, 512, 1024, 2048, 4096]
            for ci in range(len(bounds) - 1):
                lo, hi = bounds[ci], bounds[ci + 1]
                nc.sync.dma_start(lhsT[:, lo:hi], lhsT_d.ap()[:, lo:hi])
                nc.sync.dma_start(rhs[:, lo:hi], rhs_d.ap()[:, lo:hi])
            nc.sync.dma_start(ident[:], ident_d.ap())
            nc.vector.memset(ones[:], 1.0)
            nc.vector.memset(epsc[:], EPS)

            QB = 4  # row blocks per fold-chain batch
            for pb in range(NB // QB):
                quad = spool.tile([P, QB * N], DT.bfloat16, tag="strip")
                for u in range(QB):
                    bi = QB * pb + u
                    lhs_blk = lhsT[:, bi * P:(bi + 1) * P]
                    for h in range(2):
                        pt = ppool.tile([P, H], DT.float32, tag="mm")
                        for q in range(H // MM_N):
                            off = h * H + q * MM_N
                            nc.tensor.matmul(
                                pt[:, q * MM_N:(q + 1) * MM_N],
                                lhs_blk,
                                rhs[:, off:off + MM_N],
                                start=True,
                                stop=True,
                            )
                        sl = (u * 2 + h) * H
                        nc.scalar.copy(quad[:, sl:sl + H], pt[:])
                    # running column-min (per-column over row blocks), bf16 2x
                    # (first block initializes colacc with a 4x-rate copy)
                    if pb == 0 and u == 0:
                        nc.vector.tensor_copy(colacc[:], quad[:, 0:N])
                    else:
                        nc.vector.tensor_tensor(
                            out=colacc[:], in0=colacc[:],
                            in1=quad[:, u * N:(u + 1) * N], op=ALU.min)

                # row-min for QB blocks at once: bf16 pairwise-min folds at
                # 2x on 3D APs (outer dim = which block), then one 1x reduce
                w = N
                src = quad
                fv = quad[:].rearrange("p (b x) -> p b x", b=QB)
                for lvl in range(5):
                    w //= 2
                    f = scrpool.tile([P, QB * w], DT.bfloat16, tag=f"f{lvl}")
                    nc.vector.tensor_tensor(
                        out=f[:].rearrange("p (b x) -> p b x", b=QB),
                        in0=fv[:, :, 0:w], in1=fv[:, :, w:2 * w], op=ALU.min)
                    fv = f[:].rearrange("p (b x) -> p b x", b=QB)
                nc.vector.tensor_reduce(
                    out=rowacc[:, QB * pb:QB * (pb + 1)],
                    in_=fv, axis=AX.X, op=ALU.min)

            # column-min partition reduction: transpose 128x128 chunks on PE,
            # 16 chunks per PSUM tile, then one batched 3D min-reduce per tile
            G = 16
            for g in range(NB // G):
                tp = ppool.tile([P, G * P], DT.bfloat16, tag="mm")
                for c in range(G):
                    nc.tensor.transpose(
                        tp[:, c * P:(c + 1) * P],
                        colacc[:, (g * G + c) * P:(g * G + c + 1) * P], ident[:])
                nc.vector.tensor_reduce(
                    out=colminT[:, g * G:(g + 1) * G],
                    in_=tp[:].rearrange("p (n c) -> p n c", c=P),
                    axis=AX.X, op=ALU.min)

            # dist = sqrt(eps + relu(sqmin)); then mean over both directions
            d_row = cpool.tile([P, NB], DT.float32, tag="d_row")
            d_col = cpool.tile([P, NB], DT.float32, tag="d_col")
            nc.vector.tensor_scalar(
                out=d_row[:], in0=rowacc[:], scalar1=0.0, scalar2=None, op0=ALU.max)
            nc.vector.tensor_scalar(
                out=d_col[:], in0=colminT[:], scalar1=0.0, scalar2=None, op0=ALU.max)
            nc.scalar.activation(d_row[:], d_row[:], AF.Sqrt, bias=epsc[:])
            nc.scalar.activation(d_col[:], d_col[:], AF.Sqrt, bias=epsc[:])

            s1 = cpool.tile([P, 1], DT.float32, tag="s1")
            s2 = cpool.tile([P, 1], DT.float32, tag="s2")
            nc.vector.reduce_sum(out=s1[:], in_=d_row[:], axis=AX.X)
            nc.vector.reduce_sum(out=s2[:], in_=d_col[:], axis=AX.X)
            nc.vector.tensor_tensor(out=s1[:], in0=s1[:], in1=s2[:], op=ALU.add)

            pfin = ppool.tile([1, 1], DT.float32, tag="mm")
            nc.tensor.matmul(pfin[:], s1[:], ones[:], start=True, stop=True)
            res = cpool.tile([1, 1], DT.float32, tag="res")
            nc.scalar.mul(res[:], pfin[:], 1.0 / N)
            nc.sync.dma_start(out_d.ap(), res[:])

    return {"lhsT": "lhst_in", "rhs": "rhs_in", "ident": "ident_in",
            "out": "loss_out"}


@functools.lru_cache(maxsize=1)
def build_program():
    nc = bacc.Bacc("TRN2", target_bir_lowering=False, debug=False)
    names = _emit(nc)
    nc.compile()
    return nc, names


def _split(v, levels):
    outs = []
    r = v.astype(np.float64)
    for _ in range(levels):
        s = r.astype(F32).astype(BF16)
        outs.append(s)
        r = r - s.astype(np.float64)
    return outs


# (y-split, x-split) product terms kept; a+b<=2 drops only O(2^-27) terms
_PAIRS = [(0, 0), (0, 1), (1, 0), (1, 1), (0, 2), (2, 0)]


def pack_inputs(x, y):
    """Per-sample packed (lhsT, rhs) bf16 [K, N] operand pair."""
    ys = _split(y, 3)
    xs = _split(x, 3)
    m2x = [(-2.0 * s.astype(F32)).astype(BF16) for s in xs]
    y2 = (y.astype(np.float64) ** 2).sum(1).astype(F32)
    x2 = (x.astype(np.float64) ** 2).sum(1).astype(F32)
    one = np.ones(N, dtype=BF16)
    lrows, rrows = [], []
    for a, b in _PAIRS:
        for c in range(3):
            lrows.append(ys[a][:, c])
            rrows.append(m2x[b][:, c])
    for s in _split(y2, 3):
        lrows.append(s)
        rrows.append(one)
    for s in _split(x2, 3):
        lrows.append(one)
        rrows.append(s)
    lhsT = np.stack(lrows).astype(BF16)
    rhs = np.stack(rrows).astype(BF16)
    assert lhsT.shape == (K, N) and rhs.shape == (K, N)
    return np.ascontiguousarray(lhsT), np.ascontiguousarray(rhs)


def make_in_maps(x, y):
    nc, names = build_program()
    ident = np.eye(P, dtype=BF16)
    in_maps = []
    for b in range(x.shape[0]):
        lhsT, rhs = pack_inputs(np.asarray(x[b]), np.asarray(y[b]))
        in_maps.append({names["lhsT"]: lhsT, names["rhs"]: rhs,
                        names["ident"]: ident})
    return nc, names, in_maps


def run(x, y, trace=False):
    nc, names, in_maps = make_in_maps(x, y)
    res = bass_utils.run_bass_kernel_spmd(
        nc, in_maps, core_ids=list(range(len(in_maps))), trace=trace)
    out = np.array([res.results[b][names["out"]][0, 0]
                    for b in range(len(in_maps))], dtype=F32)
    return out, res


def kernel(x, y):
    out, _ = run(np.asarray(x, dtype=F32), np.asarray(y, dtype=F32))
    return out


# revision 6
# speedup vs baseline: 1.0116x; 1.0116x over previous
"""Chamfer loss Trainium2 kernel.

Per-sample Chamfer loss over (bs=8, n=4096, d=3) point clouds, data-parallel
over the batch axis: one sample per NeuronCore, no cross-core communication.

Math: dist[i,j] = sqrt(eps + relu(||y_i||^2 + ||x_j||^2 - 2 y_i.x_j)).
sqrt(eps + relu(.)) is monotonic, so min-reduce the *squared* matrix and apply
the transform to the reduced 4096-vectors only.

The squared-distance matrix is produced on the TensorEngine as a single K=24
bf16 matmul per tile: y/x are split hi+lo in bf16 (y ~ y0+y1), the squared
norms into three bf16 addends, and all product terms are stacked along the
contraction axis. PSUM accumulates in fp32, giving |sq - exact| ~ 3e-4, i.e.
~1e-5 relative error on the final loss. bf16 streams 1 cycle/row vs fp32's 4.

Per 128-row block (32 of them):
  PE    : 8 matmuls (N=512) into two [128,2048] fp32 PSUM tiles
  ACT   : copies each PSUM tile to an SBUF bf16 strip (frees PSUM, enables
          2x/4x-rate bf16 DVE ops)
  DVE   : running column-min (elementwise bf16 tensor_tensor min into
          colacc[128,4096], 2x mode) and row-min via a bf16 tensor_tensor
          min fold chain 4096->256 (2x mode) + one 1x-rate reduce
          (tensor_scalar's min-accumulate measures 1x on HW, so folds win)
Epilogue: colacc partition-min via PE transpose + DVE min-reduce per 128-col
chunk, then relu/+eps/sqrt on the two [128,32] min matrices, sum-reduce, a
ones-vector matmul for the partition sum, scale by 1/4096.

The input DMA is issued in geometric chunks (first 256 cols, then 256, 512,
1024, 2048) so the first matmul's operands land ~5us sooner than with
quarter-split DMAs; the whole kernel is start-latency + DVE-busy bound.
"""

import os
import sys
import functools

for _p in ("/opt/trn_rl_repo", "/root/.axon_site/_ro/trn_rl_repo"):
    if os.path.isdir(_p) and _p not in sys.path:
        sys.path.insert(0, _p)

import numpy as np
import ml_dtypes

import concourse.bass as bass
import concourse.bacc as bacc
import concourse.mybir as mybir
import concourse.tile as tile
from concourse import bass_utils

BF16 = ml_dtypes.bfloat16
F32 = np.float32

N = 4096          # points per cloud
P = 128           # partitions
NB = N // P       # 32 row blocks
H = 2048          # strip width (half of N), 4 PSUM banks
K = 24            # stacked contraction rows
MM_N = 512        # moving free dim per matmul (TRN2 ISA cap)
EPS = 1e-6
BIG = 1e30

AF = mybir.ActivationFunctionType
ALU = mybir.AluOpType
AX = mybir.AxisListType
DT = mybir.dt



def _emit(nc):
    lhsT_d = nc.dram_tensor("lhst_in", [K, N], DT.bfloat16, kind="ExternalInput")
    rhs_d = nc.dram_tensor("rhs_in", [K, N], DT.bfloat16, kind="ExternalInput")
    ident_d = nc.dram_tensor("ident_in", [P, P], DT.bfloat16, kind="ExternalInput")
    out_d = nc.dram_tensor("loss_out", [1, 1], DT.float32, kind="ExternalOutput")

    with tile.TileContext(nc) as tc:
        with (
            tc.tile_pool(name="const", bufs=1) as cpool,
            tc.tile_pool(name="strip", bufs=2) as spool,
            tc.tile_pool(name="scr", bufs=2) as scrpool,
            tc.tile_pool(name="psum", bufs=2, space="PSUM") as ppool,
        ):
            lhsT = cpool.tile([K, N], DT.bfloat16, tag="lhsT")
            rhs = cpool.tile([K, N], DT.bfloat16, tag="rhs")
            ident = cpool.tile([P, P], DT.bfloat16, tag="ident")
            colacc = cpool.tile([P, N], DT.bfloat16, tag="colacc")
            rowacc = cpool.tile([P, NB], DT.float32, tag="rowacc")
            colminT = cpool.tile([P, NB], DT.float32, tag="colminT")
            ones = cpool.tile([P, 1], DT.float32, tag="ones")
            epsc = cpool.tile([P, 1], DT.float32, tag="epsc")

            # input DMA ordered by first need: the first matmul reads only
            # lhsT[:, 0:128] and rhs[:, 0:512]; everything else arrives in
            # two big transfers well before block 1 is reached
            nc.sync.dma_start(lhsT[:, 0:P], lhsT_d.ap()[:, 0:P])
            nc.sync.dma_start(rhs[:, 0:MM_N], rhs_d.ap()[:, 0:MM_N])
            nc.sync.dma_start(rhs[:, MM_N:N], rhs_d.ap()[:, MM_N:N])
            nc.sync.dma_start(lhsT[:, P:N], lhsT_d.ap()[:, P:N])
            nc.sync.dma_start(ident[:], ident_d.ap())
            nc.vector.memset(ones[:], 1.0)
            nc.vector.memset(epsc[:], EPS)

            QB = 4  # row blocks per fold-chain batch
            for pb in range(NB // QB):
                quad = spool.tile([P, QB * N], DT.bfloat16, tag="strip")
                for u in range(QB):
                    bi = QB * pb + u
                    lhs_blk = lhsT[:, bi * P:(bi + 1) * P]
                    for h in range(2):
                        pt = ppool.tile([P, H], DT.float32, tag="mm")
                        for q in range(H // MM_N):
                            off = h * H + q * MM_N
                            nc.tensor.matmul(
                                pt[:, q * MM_N:(q + 1) * MM_N],
                                lhs_blk,
                                rhs[:, off:off + MM_N],
                                start=True,
                                stop=True,
                            )
                        sl = (u * 2 + h) * H
                        if pb == 0 and u == 0:
                            # block 0 fast path: drain in quarters and start
                            # the colacc chain (4x-rate init copies) as soon
                            # as each quarter lands; shaves ~8us of DVE start
                            # latency off the whole kernel
                            hq = H // 2
                            for qq in range(2):
                                nc.scalar.copy(
                                    quad[:, sl + qq * hq:sl + (qq + 1) * hq],
                                    pt[:, qq * hq:(qq + 1) * hq])
                                nc.vector.tensor_copy(
                                    colacc[:, sl + qq * hq:sl + (qq + 1) * hq],
                                    quad[:, sl + qq * hq:sl + (qq + 1) * hq])
                        else:
                            nc.scalar.copy(quad[:, sl:sl + H], pt[:])
                    # running column-min (per-column over row blocks), bf16 2x
                    if not (pb == 0 and u == 0):
                        nc.vector.tensor_tensor(
                            out=colacc[:], in0=colacc[:],
                            in1=quad[:, u * N:(u + 1) * N], op=ALU.min)

                # row-min for QB blocks at once: bf16 pairwise-min folds at
                # 2x on 3D APs (outer dim = which block), then one 1x reduce
                w = N
                src = quad
                fv = quad[:].rearrange("p (b x) -> p b x", b=QB)
                for lvl in range(6):
                    w //= 2
                    f = scrpool.tile([P, QB * w], DT.bfloat16, tag=f"f{lvl}")
                    nc.vector.tensor_tensor(
                        out=f[:].rearrange("p (b x) -> p b x", b=QB),
                        in0=fv[:, :, 0:w], in1=fv[:, :, w:2 * w], op=ALU.min)
                    fv = f[:].rearrange("p (b x) -> p b x", b=QB)
                nc.vector.tensor_reduce(
                    out=rowacc[:, QB * pb:QB * (pb + 1)],
                    in_=fv, axis=AX.X, op=ALU.min)

            # column-min partition reduction: transpose 128x128 chunks on PE,
            # 16 chunks per PSUM tile, then one batched 3D min-reduce per tile
            G = 16
            for g in range(NB // G):
                tp = ppool.tile([P, G * P], DT.bfloat16, tag="mm")
                for c in range(G):
                    nc.tensor.transpose(
                        tp[:, c * P:(c + 1) * P],
                        colacc[:, (g * G + c) * P:(g * G + c + 1) * P], ident[:])
                nc.vector.tensor_reduce(
                    out=colminT[:, g * G:(g + 1) * G],
                    in_=tp[:].rearrange("p (n c) -> p n c", c=P),
                    axis=AX.X, op=ALU.min)

            # dist = sqrt(eps + relu(sqmin)); then mean over both directions
            d_row = cpool.tile([P, NB], DT.float32, tag="d_row")
            d_col = cpool.tile([P, NB], DT.float32, tag="d_col")
            nc.vector.tensor_scalar(
                out=d_row[:], in0=rowacc[:], scalar1=0.0, scalar2=None, op0=ALU.max)
            nc.vector.tensor_scalar(
                out=d_col[:], in0=colminT[:], scalar1=0.0, scalar2=None, op0=ALU.max)
            nc.scalar.activation(d_row[:], d_row[:], AF.Sqrt, bias=epsc[:])
            nc.scalar.activation(d_col[:], d_col[:], AF.Sqrt, bias=epsc[:])

            s1 = cpool.tile([P, 1], DT.float32, tag="s1")
            s2 = cpool.tile([P, 1], DT.float32, tag="s2")
            nc.vector.reduce_sum(out=s1[:], in_=d_row[:], axis=AX.X)
            nc.vector.reduce_sum(out=s2[:], in_=d_col[:], axis=AX.X)
            nc.vector.tensor_tensor(out=s1[:], in0=s1[:], in1=s2[:], op=ALU.add)

            pfin = ppool.tile([1, 1], DT.float32, tag="mm")
            nc.tensor.matmul(pfin[:], s1[:], ones[:], start=True, stop=True)
            res = cpool.tile([1, 1], DT.float32, tag="res")
            nc.scalar.mul(res[:], pfin[:], 1.0 / N)
            nc.sync.dma_start(out_d.ap(), res[:])

    return {"lhsT": "lhst_in", "rhs": "rhs_in", "ident": "ident_in",
            "out": "loss_out"}


@functools.lru_cache(maxsize=1)
def build_program():
    nc = bacc.Bacc("TRN2", target_bir_lowering=False, debug=False)
    names = _emit(nc)
    nc.compile()
    return nc, names


def _split(v, levels):
    outs = []
    r = v.astype(np.float64)
    for _ in range(levels):
        s = r.astype(F32).astype(BF16)
        outs.append(s)
        r = r - s.astype(np.float64)
    return outs


# (y-split, x-split) product terms kept; a+b<=2 drops only O(2^-27) terms
_PAIRS = [(0, 0), (0, 1), (1, 0), (1, 1), (0, 2), (2, 0)]


def pack_inputs(x, y):
    """Per-sample packed (lhsT, rhs) bf16 [K, N] operand pair."""
    ys = _split(y, 3)
    xs = _split(x, 3)
    m2x = [(-2.0 * s.astype(F32)).astype(BF16) for s in xs]
    y2 = (y.astype(np.float64) ** 2).sum(1).astype(F32)
    x2 = (x.astype(np.float64) ** 2).sum(1).astype(F32)
    one = np.ones(N, dtype=BF16)
    lrows, rrows = [], []
    for a, b in _PAIRS:
        for c in range(3):
            lrows.append(ys[a][:, c])
            rrows.append(m2x[b][:, c])
    for s in _split(y2, 3):
        lrows.append(s)
        rrows.append(one)
    for s in _split(x2, 3):
        lrows.append(one)
        rrows.append(s)
    lhsT = np.stack(lrows).astype(BF16)
    rhs = np.stack(rrows).astype(BF16)
    assert lhsT.shape == (K, N) and rhs.shape == (K, N)
    return np.ascontiguousarray(lhsT), np.ascontiguousarray(rhs)


def make_in_maps(x, y):
    nc, names = build_program()
    ident = np.eye(P, dtype=BF16)
    in_maps = []
    for b in range(x.shape[0]):
        lhsT, rhs = pack_inputs(np.asarray(x[b]), np.asarray(y[b]))
        in_maps.append({names["lhsT"]: lhsT, names["rhs"]: rhs,
                        names["ident"]: ident})
    return nc, names, in_maps


def run(x, y, trace=False):
    nc, names, in_maps = make_in_maps(x, y)
    res = bass_utils.run_bass_kernel_spmd(
        nc, in_maps, core_ids=list(range(len(in_maps))), trace=trace)
    out = np.array([res.results[b][names["out"]][0, 0]
                    for b in range(len(in_maps))], dtype=F32)
    return out, res


def kernel(x, y):
    out, _ = run(np.asarray(x, dtype=F32), np.asarray(y, dtype=F32))
    return out


# revision 10
# speedup vs baseline: 1.0144x; 1.0027x over previous
"""Chamfer loss Trainium2 kernel.

Per-sample Chamfer loss over (bs=8, n=4096, d=3) point clouds, data-parallel
over the batch axis: one sample per NeuronCore, no cross-core communication.

Math: dist[i,j] = sqrt(eps + relu(||y_i||^2 + ||x_j||^2 - 2 y_i.x_j)).
sqrt(eps + relu(.)) is monotonic, so min-reduce the *squared* matrix and apply
the transform to the reduced 4096-vectors only.

The squared-distance matrix is produced on the TensorEngine as a single K=24
bf16 matmul per tile: y/x are split hi+lo in bf16 (y ~ y0+y1), the squared
norms into three bf16 addends, and all product terms are stacked along the
contraction axis. PSUM accumulates in fp32, giving |sq - exact| ~ 3e-4, i.e.
~1e-5 relative error on the final loss. bf16 streams 1 cycle/row vs fp32's 4.

Per 128-row block (32 of them):
  PE    : 8 matmuls (N=512) into two [128,2048] fp32 PSUM tiles
  ACT   : copies each PSUM tile to an SBUF bf16 strip (frees PSUM, enables
          2x/4x-rate bf16 DVE ops)
  DVE   : running column-min (elementwise bf16 tensor_tensor min into
          colacc[128,4096], 2x mode) and row-min via a bf16 tensor_tensor
          min fold chain 4096->256 (2x mode) + one 1x-rate reduce
          (tensor_scalar's min-accumulate measures 1x on HW, so folds win)
Epilogue: colacc partition-min via PE transpose + DVE min-reduce per 128-col
chunk, then relu/+eps/sqrt on the two [128,32] min matrices, sum-reduce, a
ones-vector matmul for the partition sum, scale by 1/4096.

The input DMA is issued in geometric chunks (first 256 cols, then 256, 512,
1024, 2048) so the first matmul's operands land ~5us sooner than with
quarter-split DMAs; the whole kernel is start-latency + DVE-busy bound.
"""

import os
import sys
import functools

for _p in ("/opt/trn_rl_repo", "/root/.axon_site/_ro/trn_rl_repo"):
    if os.path.isdir(_p) and _p not in sys.path:
        sys.path.insert(0, _p)

import numpy as np
import ml_dtypes

import concourse.bass as bass
import concourse.bacc as bacc
import concourse.mybir as mybir
import concourse.tile as tile
from concourse import bass_utils

BF16 = ml_dtypes.bfloat16
F32 = np.float32

N = 4096          # points per cloud
P = 128           # partitions
NB = N // P       # 32 row blocks
H = 2048          # strip width (half of N), 4 PSUM banks
K = 24            # stacked contraction rows
MM_N = 512        # moving free dim per matmul (TRN2 ISA cap)
EPS = 1e-6
BIG = 1e30

AF = mybir.ActivationFunctionType
ALU = mybir.AluOpType
AX = mybir.AxisListType
DT = mybir.dt



def _emit(nc):
    lhsT_d = nc.dram_tensor("lhst_in", [K, N], DT.bfloat16, kind="ExternalInput")
    rhs_d = nc.dram_tensor("rhs_in", [K, N], DT.bfloat16, kind="ExternalInput")
    ident_d = nc.dram_tensor("ident_in", [P, P], DT.bfloat16, kind="ExternalInput")
    out_d = nc.dram_tensor("loss_out", [1, 1], DT.float32, kind="ExternalOutput")

    with tile.TileContext(nc) as tc:
        with (
            tc.tile_pool(name="const", bufs=1) as cpool,
            tc.tile_pool(name="strip", bufs=2) as spool,
            tc.tile_pool(name="scr", bufs=2) as scrpool,
            tc.tile_pool(name="psum", bufs=2, space="PSUM") as ppool,
        ):
            lhsT = cpool.tile([K, N], DT.bfloat16, tag="lhsT")
            rhs = cpool.tile([K, N], DT.bfloat16, tag="rhs")
            ident = cpool.tile([P, P], DT.bfloat16, tag="ident")
            colacc = cpool.tile([P, N], DT.bfloat16, tag="colacc")
            rowacc = cpool.tile([P, NB], DT.float32, tag="rowacc")
            colminT = cpool.tile([P, NB], DT.float32, tag="colminT")
            ones = cpool.tile([P, 1], DT.float32, tag="ones")
            epsc = cpool.tile([P, 1], DT.float32, tag="epsc")

            # input DMA ordered by first need: the first matmul reads only
            # lhsT[:, 0:128] and rhs[:, 0:512]; everything else arrives in
            # two big transfers well before block 1 is reached
            nc.sync.dma_start(lhsT[:, 0:P], lhsT_d.ap()[:, 0:P])
            nc.sync.dma_start(rhs[:, 0:MM_N], rhs_d.ap()[:, 0:MM_N])
            nc.sync.dma_start(rhs[:, MM_N:N], rhs_d.ap()[:, MM_N:N])
            nc.sync.dma_start(lhsT[:, P:N], lhsT_d.ap()[:, P:N])
            nc.sync.dma_start(ident[:], ident_d.ap())
            nc.vector.memset(ones[:], 1.0)
            nc.vector.memset(epsc[:], EPS)
            # preload the sqrt activation table so the epilogue doesn't pay
            # the ~1.3us ACT_TABLE_LOAD on the critical tail
            warm = cpool.tile([P, 1], DT.float32, tag="warm")
            nc.scalar.activation(warm[:], ones[:], AF.Sqrt, bias=epsc[:])

            QB = 4  # row blocks per fold-chain batch
            for pb in range(NB // QB):
                quad = spool.tile([P, QB * N], DT.bfloat16, tag="strip")
                for u in range(QB):
                    bi = QB * pb + u
                    lhs_blk = lhsT[:, bi * P:(bi + 1) * P]
                    for h in range(2):
                        pt = ppool.tile([P, H], DT.float32, tag="mm")
                        for q in range(H // MM_N):
                            off = h * H + q * MM_N
                            nc.tensor.matmul(
                                pt[:, q * MM_N:(q + 1) * MM_N],
                                lhs_blk,
                                rhs[:, off:off + MM_N],
                                start=True,
                                stop=True,
                            )
                        sl = (u * 2 + h) * H
                        if pb == 0 and u == 0:
                            # block 0 fast path: drain in quarters and start
                            # the colacc chain (4x-rate init copies) as soon
                            # as each quarter lands; shaves ~8us of DVE start
                            # latency off the whole kernel
                            hq = H // 2
                            for qq in range(2):
                                nc.scalar.copy(
                                    quad[:, sl + qq * hq:sl + (qq + 1) * hq],
                                    pt[:, qq * hq:(qq + 1) * hq])
                                nc.vector.tensor_copy(
                                    colacc[:, sl + qq * hq:sl + (qq + 1) * hq],
                                    quad[:, sl + qq * hq:sl + (qq + 1) * hq])
                        else:
                            nc.scalar.copy(quad[:, sl:sl + H], pt[:])
                    # running column-min (per-column over row blocks), bf16 2x
                    # (early blocks: two half-width TTs so the chain can start
                    # right after the first half-strip drain — the pipeline is
                    # still filling there and DVE would otherwise idle)
                    if not (pb == 0 and u == 0):
                        bi_g = QB * pb + u
                        if bi_g < 8:
                            for hh in range(2):
                                nc.vector.tensor_tensor(
                                    out=colacc[:, hh * H:(hh + 1) * H],
                                    in0=colacc[:, hh * H:(hh + 1) * H],
                                    in1=quad[:, u * N + hh * H:u * N + (hh + 1) * H],
                                    op=ALU.min)
                        else:
                            nc.vector.tensor_tensor(
                                out=colacc[:], in0=colacc[:],
                                in1=quad[:, u * N:(u + 1) * N], op=ALU.min)

                # row-min for QB blocks at once: bf16 pairwise-min folds at
                # 2x on 3D APs (outer dim = which block), then one 1x reduce
                w = N
                src = quad
                fv = quad[:].rearrange("p (b x) -> p b x", b=QB)
                for lvl in range(6):
                    w //= 2
                    f = scrpool.tile([P, QB * w], DT.bfloat16, tag=f"f{lvl}")
                    nc.vector.tensor_tensor(
                        out=f[:].rearrange("p (b x) -> p b x", b=QB),
                        in0=fv[:, :, 0:w], in1=fv[:, :, w:2 * w], op=ALU.min)
                    fv = f[:].rearrange("p (b x) -> p b x", b=QB)
                nc.vector.tensor_reduce(
                    out=rowacc[:, QB * pb:QB * (pb + 1)],
                    in_=fv, axis=AX.X, op=ALU.min)

            # dist = sqrt(eps + relu(sqmin)): do the row direction first so
            # DVE/ACT have work while the PE transposes colacc chunks below
            d_row = cpool.tile([P, NB], DT.float32, tag="d_row")
            d_col = cpool.tile([P, NB], DT.float32, tag="d_col")
            s1 = cpool.tile([P, 1], DT.float32, tag="s1")
            s2 = cpool.tile([P, 1], DT.float32, tag="s2")
            nc.vector.tensor_scalar(
                out=d_row[:], in0=rowacc[:], scalar1=0.0, scalar2=None, op0=ALU.max)
            nc.scalar.activation(d_row[:], d_row[:], AF.Sqrt, bias=epsc[:])
            nc.vector.reduce_sum(out=s1[:], in_=d_row[:], axis=AX.X)

            # column-min partition reduction: transpose 128x128 chunks on PE,
            # 8 chunks per PSUM tile, then one batched 3D min-reduce per tile
            # (small groups keep only the last group's reduce on the tail)
            G = 8
            for g in range(NB // G):
                tp = ppool.tile([P, G * P], DT.bfloat16, tag="mm")
                for c in range(G):
                    nc.tensor.transpose(
                        tp[:, c * P:(c + 1) * P],
                        colacc[:, (g * G + c) * P:(g * G + c + 1) * P], ident[:])
                nc.vector.tensor_reduce(
                    out=colminT[:, g * G:(g + 1) * G],
                    in_=tp[:].rearrange("p (n c) -> p n c", c=P),
                    axis=AX.X, op=ALU.min)

            nc.vector.tensor_scalar(
                out=d_col[:], in0=colminT[:], scalar1=0.0, scalar2=None, op0=ALU.max)
            nc.scalar.activation(d_col[:], d_col[:], AF.Sqrt, bias=epsc[:])
            nc.vector.reduce_sum(out=s2[:], in_=d_col[:], axis=AX.X)
            nc.vector.tensor_tensor(out=s1[:], in0=s1[:], in1=s2[:], op=ALU.add)

            pfin = ppool.tile([1, 1], DT.float32, tag="mm")
            nc.tensor.matmul(pfin[:], s1[:], ones[:], start=True, stop=True)
            res = cpool.tile([1, 1], DT.float32, tag="res")
            nc.scalar.mul(res[:], pfin[:], 1.0 / N)
            nc.sync.dma_start(out_d.ap(), res[:])

    return {"lhsT": "lhst_in", "rhs": "rhs_in", "ident": "ident_in",
            "out": "loss_out"}


@functools.lru_cache(maxsize=1)
def build_program():
    nc = bacc.Bacc("TRN2", target_bir_lowering=False, debug=False)
    names = _emit(nc)
    nc.compile()
    return nc, names


def _split(v, levels):
    outs = []
    r = v.astype(np.float64)
    for _ in range(levels):
        s = r.astype(F32).astype(BF16)
        outs.append(s)
        r = r - s.astype(np.float64)
    return outs


# (y-split, x-split) product terms kept; a+b<=2 drops only O(2^-27) terms
_PAIRS = [(0, 0), (0, 1), (1, 0), (1, 1), (0, 2), (2, 0)]


def pack_inputs(x, y):
    """Per-sample packed (lhsT, rhs) bf16 [K, N] operand pair."""
    ys = _split(y, 3)
    xs = _split(x, 3)
    m2x = [(-2.0 * s.astype(F32)).astype(BF16) for s in xs]
    y2 = (y.astype(np.float64) ** 2).sum(1).astype(F32)
    x2 = (x.astype(np.float64) ** 2).sum(1).astype(F32)
    one = np.ones(N, dtype=BF16)
    lrows, rrows = [], []
    for a, b in _PAIRS:
        for c in range(3):
            lrows.append(ys[a][:, c])
            rrows.append(m2x[b][:, c])
    for s in _split(y2, 3):
        lrows.append(s)
        rrows.append(one)
    for s in _split(x2, 3):
        lrows.append(one)
        rrows.append(s)
    lhsT = np.stack(lrows).astype(BF16)
    rhs = np.stack(rrows).astype(BF16)
    assert lhsT.shape == (K, N) and rhs.shape == (K, N)
    return np.ascontiguousarray(lhsT), np.ascontiguousarray(rhs)


def make_in_maps(x, y):
    nc, names = build_program()
    ident = np.eye(P, dtype=BF16)
    in_maps = []
    for b in range(x.shape[0]):
        lhsT, rhs = pack_inputs(np.asarray(x[b]), np.asarray(y[b]))
        in_maps.append({names["lhsT"]: lhsT, names["rhs"]: rhs,
                        names["ident"]: ident})
    return nc, names, in_maps


def run(x, y, trace=False):
    nc, names, in_maps = make_in_maps(x, y)
    res = bass_utils.run_bass_kernel_spmd(
        nc, in_maps, core_ids=list(range(len(in_maps))), trace=trace)
    out = np.array([res.results[b][names["out"]][0, 0]
                    for b in range(len(in_maps))], dtype=F32)
    return out, res


def kernel(x, y):
    out, _ = run(np.asarray(x, dtype=F32), np.asarray(y, dtype=F32))
    return out


# revision 13
# speedup vs baseline: 1.0241x; 1.0096x over previous
"""Chamfer loss Trainium2 kernel.

Per-sample Chamfer loss over (bs=8, n=4096, d=3) point clouds, data-parallel
over the batch axis: one sample per NeuronCore, no cross-core communication.

Math: dist[i,j] = sqrt(eps + relu(||y_i||^2 + ||x_j||^2 - 2 y_i.x_j)).
sqrt(eps + relu(.)) is monotonic, so min-reduce the *squared* matrix and apply
the transform to the reduced 4096-vectors only.

The squared-distance matrix is produced on the TensorEngine as a single K=24
bf16 matmul per tile: y/x are split hi+lo in bf16 (y ~ y0+y1), the squared
norms into three bf16 addends, and all product terms are stacked along the
contraction axis. PSUM accumulates in fp32, giving |sq - exact| ~ 3e-4, i.e.
~1e-5 relative error on the final loss. bf16 streams 1 cycle/row vs fp32's 4.

Per 128-row block (32 of them):
  PE    : 8 matmuls (N=512) into two [128,2048] fp32 PSUM tiles
  ACT   : copies each PSUM tile to an SBUF bf16 strip (frees PSUM, enables
          2x/4x-rate bf16 DVE ops)
  DVE   : running column-min (elementwise bf16 tensor_tensor min into
          colacc[128,4096], 2x mode) and row-min via a bf16 tensor_tensor
          min fold chain 4096->256 (2x mode) + one 1x-rate reduce
          (tensor_scalar's min-accumulate measures 1x on HW, so folds win)
Epilogue: colacc partition-min via PE transpose + DVE min-reduce per 128-col
chunk, then relu/+eps/sqrt on the two [128,32] min matrices, sum-reduce, a
ones-vector matmul for the partition sum, scale by 1/4096.

The input DMA is issued in geometric chunks (first 256 cols, then 256, 512,
1024, 2048) so the first matmul's operands land ~5us sooner than with
quarter-split DMAs; the whole kernel is start-latency + DVE-busy bound.
"""

import os
import sys
import functools

for _p in ("/opt/trn_rl_repo", "/root/.axon_site/_ro/trn_rl_repo"):
    if os.path.isdir(_p) and _p not in sys.path:
        sys.path.insert(0, _p)

import numpy as np
import ml_dtypes

import concourse.bass as bass
import concourse.bacc as bacc
import concourse.mybir as mybir
import concourse.tile as tile
from concourse import bass_utils

BF16 = ml_dtypes.bfloat16
F32 = np.float32

N = 4096          # points per cloud
P = 128           # partitions
NB = N // P       # 32 row blocks
H = 2048          # strip width (half of N), 4 PSUM banks
K = 24            # stacked contraction rows
MM_N = 512        # moving free dim per matmul (TRN2 ISA cap)
EPS = 1e-6
BIG = 1e30

AF = mybir.ActivationFunctionType
ALU = mybir.AluOpType
AX = mybir.AxisListType
DT = mybir.dt



def _emit(nc):
    lhsT_d = nc.dram_tensor("lhst_in", [K, N], DT.bfloat16, kind="ExternalInput")
    rhs_d = nc.dram_tensor("rhs_in", [K, N], DT.bfloat16, kind="ExternalInput")
    ident_d = nc.dram_tensor("ident_in", [P, P], DT.bfloat16, kind="ExternalInput")
    out_d = nc.dram_tensor("loss_out", [1, 1], DT.float32, kind="ExternalOutput")

    with tile.TileContext(nc) as tc:
        with (
            tc.tile_pool(name="const", bufs=1) as cpool,
            tc.tile_pool(name="strip", bufs=2) as spool,
            tc.tile_pool(name="scr", bufs=2) as scrpool,
            tc.tile_pool(name="psum", bufs=2, space="PSUM") as ppool,
        ):
            lhsT = cpool.tile([K, N], DT.bfloat16, tag="lhsT")
            rhs = cpool.tile([K, N], DT.bfloat16, tag="rhs")
            ident = cpool.tile([P, P], DT.bfloat16, tag="ident")
            colacc = cpool.tile([P, N], DT.bfloat16, tag="colacc")
            rowacc = cpool.tile([P, NB], DT.float32, tag="rowacc")
            colminT = cpool.tile([P, NB], DT.float32, tag="colminT")
            ones = cpool.tile([P, 1], DT.float32, tag="ones")
            epsc = cpool.tile([P, 1], DT.float32, tag="epsc")

            # input DMA ordered by first need: the first matmul reads only
            # lhsT[:, 0:128] and rhs[:, 0:512]; everything else arrives in
            # two big transfers well before block 1 is reached
            nc.sync.dma_start(lhsT[:, 0:P], lhsT_d.ap()[:, 0:P])
            nc.sync.dma_start(rhs[:, 0:MM_N], rhs_d.ap()[:, 0:MM_N])
            nc.sync.dma_start(rhs[:, MM_N:N], rhs_d.ap()[:, MM_N:N])
            nc.sync.dma_start(lhsT[:, P:N], lhsT_d.ap()[:, P:N])
            nc.sync.dma_start(ident[:], ident_d.ap())
            nc.vector.memset(ones[:], 1.0)
            nc.vector.memset(epsc[:], EPS)
            # preload the sqrt activation table so the epilogue doesn't pay
            # the ~1.3us ACT_TABLE_LOAD on the critical tail
            warm = cpool.tile([P, 1], DT.float32, tag="warm")
            nc.scalar.activation(warm[:], ones[:], AF.Sqrt, bias=epsc[:])

            QB = 4  # row blocks per fold-chain batch
            for pb in range(NB // QB):
                quad = spool.tile([P, QB * N], DT.bfloat16, tag="strip")
                for u in range(QB):
                    bi = QB * pb + u
                    lhs_blk = lhsT[:, bi * P:(bi + 1) * P]
                    if pb == 0 and u == 0:
                        # block 0 fast path: separate [128,1024] PSUM tiles so
                        # each quarter drains right after its 2 matmuls (PSUM
                        # deps are per-tile), and the colacc chain starts with
                        # 4x-rate init copies as soon as each quarter lands —
                        # shaves several us of DVE start latency
                        hq = H // 2
                        for qq in range(4):
                            ptq = ppool.tile([P, hq], DT.float32, tag="mm")
                            for q in range(2):
                                off = qq * hq + q * MM_N
                                nc.tensor.matmul(
                                    ptq[:, q * MM_N:(q + 1) * MM_N],
                                    lhs_blk,
                                    rhs[:, off:off + MM_N],
                                    start=True,
                                    stop=True,
                                )
                            nc.scalar.copy(quad[:, qq * hq:(qq + 1) * hq], ptq[:])
                            nc.vector.tensor_copy(
                                colacc[:, qq * hq:(qq + 1) * hq],
                                quad[:, qq * hq:(qq + 1) * hq])
                        continue
                    for h in range(2):
                        pt = ppool.tile([P, H], DT.float32, tag="mm")
                        for q in range(H // MM_N):
                            off = h * H + q * MM_N
                            nc.tensor.matmul(
                                pt[:, q * MM_N:(q + 1) * MM_N],
                                lhs_blk,
                                rhs[:, off:off + MM_N],
                                start=True,
                                stop=True,
                            )
                        sl = (u * 2 + h) * H
                        nc.scalar.copy(quad[:, sl:sl + H], pt[:])
                    # running column-min (per-column over row blocks), bf16 2x
                    # (early blocks: two half-width TTs so the chain can start
                    # right after the first half-strip drain — the pipeline is
                    # still filling there and DVE would otherwise idle)
                    if not (pb == 0 and u == 0):
                        bi_g = QB * pb + u
                        if bi_g < 4:
                            for hh in range(2):
                                nc.vector.tensor_tensor(
                                    out=colacc[:, hh * H:(hh + 1) * H],
                                    in0=colacc[:, hh * H:(hh + 1) * H],
                                    in1=quad[:, u * N + hh * H:u * N + (hh + 1) * H],
                                    op=ALU.min)
                        else:
                            nc.vector.tensor_tensor(
                                out=colacc[:], in0=colacc[:],
                                in1=quad[:, u * N:(u + 1) * N], op=ALU.min)

                # row-min for QB blocks at once: bf16 pairwise-min folds at
                # 2x on 3D APs (outer dim = which block), then one 1x reduce
                w = N
                src = quad
                fv = quad[:].rearrange("p (b x) -> p b x", b=QB)
                for lvl in range(5):
                    w //= 2
                    f = scrpool.tile([P, QB * w], DT.bfloat16, tag=f"f{lvl}")
                    nc.vector.tensor_tensor(
                        out=f[:].rearrange("p (b x) -> p b x", b=QB),
                        in0=fv[:, :, 0:w], in1=fv[:, :, w:2 * w], op=ALU.min)
                    fv = f[:].rearrange("p (b x) -> p b x", b=QB)
                nc.vector.tensor_reduce(
                    out=rowacc[:, QB * pb:QB * (pb + 1)],
                    in_=fv, axis=AX.X, op=ALU.min)

            # dist = sqrt(eps + relu(sqmin)): do the row direction first so
            # DVE/ACT have work while the PE transposes colacc chunks below
            d_row = cpool.tile([P, NB], DT.float32, tag="d_row")
            d_col = cpool.tile([P, NB], DT.float32, tag="d_col")
            s1 = cpool.tile([P, 1], DT.float32, tag="s1")
            s2 = cpool.tile([P, 1], DT.float32, tag="s2")
            nc.vector.tensor_scalar(
                out=d_row[:], in0=rowacc[:], scalar1=0.0, scalar2=None, op0=ALU.max)
            nc.scalar.activation(d_row[:], d_row[:], AF.Sqrt, bias=epsc[:])
            nc.vector.reduce_sum(out=s1[:], in_=d_row[:], axis=AX.X)

            # column-min partition reduction: transpose 128x128 chunks on PE,
            # 8 chunks per PSUM tile, then one batched 3D min-reduce per tile
            # (small groups keep only the last group's reduce on the tail)
            G = 8
            for g in range(NB // G):
                tp = ppool.tile([P, G * P], DT.bfloat16, tag="mm")
                for c in range(G):
                    nc.tensor.transpose(
                        tp[:, c * P:(c + 1) * P],
                        colacc[:, (g * G + c) * P:(g * G + c + 1) * P], ident[:])
                nc.vector.tensor_reduce(
                    out=colminT[:, g * G:(g + 1) * G],
                    in_=tp[:].rearrange("p (n c) -> p n c", c=P),
                    axis=AX.X, op=ALU.min)

            nc.vector.tensor_scalar(
                out=d_col[:], in0=colminT[:], scalar1=0.0, scalar2=None, op0=ALU.max)
            nc.scalar.activation(d_col[:], d_col[:], AF.Sqrt, bias=epsc[:])
            nc.vector.reduce_sum(out=s2[:], in_=d_col[:], axis=AX.X)
            nc.vector.tensor_tensor(out=s1[:], in0=s1[:], in1=s2[:], op=ALU.add)

            pfin = ppool.tile([1, 1], DT.float32, tag="mm")
            nc.tensor.matmul(pfin[:], s1[:], ones[:], start=True, stop=True)
            res = cpool.tile([1, 1], DT.float32, tag="res")
            nc.scalar.mul(res[:], pfin[:], 1.0 / N)
            nc.sync.dma_start(out_d.ap(), res[:])

    return {"lhsT": "lhst_in", "rhs": "rhs_in", "ident": "ident_in",
            "out": "loss_out"}


@functools.lru_cache(maxsize=1)
def build_program():
    nc = bacc.Bacc("TRN2", target_bir_lowering=False, debug=False)
    names = _emit(nc)
    nc.compile()
    return nc, names


def _split(v, levels):
    outs = []
    r = v.astype(np.float64)
    for _ in range(levels):
        s = r.astype(F32).astype(BF16)
        outs.append(s)
        r = r - s.astype(np.float64)
    return outs


# (y-split, x-split) product terms kept; a+b<=2 drops only O(2^-27) terms
_PAIRS = [(0, 0), (0, 1), (1, 0), (1, 1), (0, 2), (2, 0)]


def pack_inputs(x, y):
    """Per-sample packed (lhsT, rhs) bf16 [K, N] operand pair."""
    ys = _split(y, 3)
    xs = _split(x, 3)
    m2x = [(-2.0 * s.astype(F32)).astype(BF16) for s in xs]
    y2 = (y.astype(np.float64) ** 2).sum(1).astype(F32)
    x2 = (x.astype(np.float64) ** 2).sum(1).astype(F32)
    one = np.ones(N, dtype=BF16)
    lrows, rrows = [], []
    for a, b in _PAIRS:
        for c in range(3):
            lrows.append(ys[a][:, c])
            rrows.append(m2x[b][:, c])
    for s in _split(y2, 3):
        lrows.append(s)
        rrows.append(one)
    for s in _split(x2, 3):
        lrows.append(one)
        rrows.append(s)
    lhsT = np.stack(lrows).astype(BF16)
    rhs = np.stack(rrows).astype(BF16)
    assert lhsT.shape == (K, N) and rhs.shape == (K, N)
    return np.ascontiguousarray(lhsT), np.ascontiguousarray(rhs)


def make_in_maps(x, y):
    nc, names = build_program()
    ident = np.eye(P, dtype=BF16)
    in_maps = []
    for b in range(x.shape[0]):
        lhsT, rhs = pack_inputs(np.asarray(x[b]), np.asarray(y[b]))
        in_maps.append({names["lhsT"]: lhsT, names["rhs"]: rhs,
                        names["ident"]: ident})
    return nc, names, in_maps


def run(x, y, trace=False):
    nc, names, in_maps = make_in_maps(x, y)
    res = bass_utils.run_bass_kernel_spmd(
        nc, in_maps, core_ids=list(range(len(in_maps))), trace=trace)
    out = np.array([res.results[b][names["out"]][0, 0]
                    for b in range(len(in_maps))], dtype=F32)
    return out, res


def kernel(x, y):
    out, _ = run(np.asarray(x, dtype=F32), np.asarray(y, dtype=F32))
    return out
